# revision 1
# baseline (speedup 1.0000x reference)
"""MuSc (Mutual Scoring) Trainium2 kernel.

Problem: nn_BatchMuSc — Z:[16,1369,1024] patch features, cls_tokens:[16,1024].
MSM: for each image i, per-patch score = mean of the 4 smallest per-image
min-distances (excluding self). Then image scores -> min-max norm -> MMO over
cls-token similarity.

Strategy (8 NeuronCores, data-parallel over query image pairs):
  - Core c owns query images (2c, 2c+1). All inputs to core c are ROTATED so
    position 0 = image 2c; self-exclusion positions are then core-invariant
    (pos 0 for local img 0, pos 1 for local img 1) => one SPMD program.
  - Host pre-transposes Z to feature-major fp16 tiles [128, 8k, 1408] per
    image (refs padded 1369->1408 with a constant vector, whose distance is
    always huge) and pre-broadcasts ref squared-norms across partitions.
  - Device, per (query image, 128-query block, ref position, 512-ref chunk):
    PSUM[q,r] = sum_k (-2*q_k)*r_k via 8 fp16 matmuls; one fused DVE
    tensor_tensor_reduce adds ref norms, min-reduces over the chunk and
    chains the running min across chunks => m[q, pos] = min d^2 - |q|^2.
  - Tail on device: 4 smallest of m row via iterative masked min; each
    + |q|^2 -> sqrt (ACT); mean -> per-patch score. Host does the tiny
    [16]-vector min-max norm + 16x16 MMO tail in float64.
"""

import os
import numpy as np

N = 16            # images
L = 1369          # patches per image
C = 1024          # feature dim
NCORES = 8
LP = 1408         # padded patches (11 * 128)
NQB = 11          # query blocks of 128
KCH = 8           # contraction chunks of 128
CHUNKS = [(0, 512), (512, 512), (1024, 345)]   # 1369 real refs; pad cols excluded
PAD_VAL = np.float16(2.0)   # pad-row feature value; pad d^2 ~ |q|^2+4096-4*sum(q) >> real min
PAD_NORM = 4096.0           # C * PAD_VAL^2
BIG = 3.0e38

_CACHE = {}


def _build():
    import concourse.bacc as bacc
    import concourse.tile as tile
    from concourse import mybir

    f16 = mybir.dt.float16
    f32 = mybir.dt.float32
    Sqrt = mybir.ActivationFunctionType.Sqrt
    Alu = mybir.AluOpType
    AxX = mybir.AxisListType.X

    nc = bacc.Bacc("TRN2", target_bir_lowering=False, debug=False)

    zt = nc.dram_tensor("zt", [N, 128, KCH, LP], f16, kind="ExternalInput").ap()
    nb = nc.dram_tensor("nb", [N, 128, LP], f32, kind="ExternalInput").ap()
    q2 = nc.dram_tensor("q2", [2, 128, NQB], f32, kind="ExternalInput").ap()
    out = nc.dram_tensor("scores", [2, 128, NQB], f32, kind="ExternalOutput").ap()

    with tile.TileContext(nc) as tc:
        with (
            tc.tile_pool(name="qpool", bufs=1) as qpool,
            tc.tile_pool(name="refpool", bufs=3) as refpool,
            tc.tile_pool(name="nbpool", bufs=3) as nbpool,
            tc.tile_pool(name="mpool", bufs=1) as mpool,
            tc.tile_pool(name="smpool", bufs=8) as smpool,
            tc.tile_pool(name="scrpool", bufs=6) as scrpool,
            tc.tile_pool(name="scorepool", bufs=1) as scorepool,
            tc.tile_pool(name="psum", bufs=7, space="PSUM") as psum,
        ):
            # resident tiles for the core's own 2 images: used as BOTH the
            # query lhsT (raw, un-scaled) and the ref tiles for positions 0/1
            qsb = []
            for i in range(2):
                t = qpool.tile([128, KCH, LP], f16, name=f"q{i}", tag=f"q{i}")
                nc.sync.dma_start(t[:], zt[i])
                qsb.append(t)
            q2sb = []
            for i in range(2):
                t = qpool.tile([128, NQB], f32, name=f"q2_{i}", tag=f"q2_{i}")
                nc.sync.dma_start(t[:], q2[i])
                q2sb.append(t)

            # persistent min accumulators m[i][qb] : [128, N] (d^2 - |q|^2 per ref pos)
            msb = [[mpool.tile([128, N], f32, name=f"m_{i}_{qb}", tag=f"m_{i}_{qb}") for qb in range(NQB)]
                   for i in range(2)]
            for i in range(2):
                for qb in range(NQB):
                    nc.vector.memset(msb[i][qb][:], -BIG)

            scoresb = [scorepool.tile([128, NQB], f32, name=f"sc{i}", tag=f"sc{i}") for i in range(2)]

            for t in [0] + list(range(2, N)) + [1]:
                if t < 2:
                    rsb = qsb[t]
                else:
                    rsb = refpool.tile([128, KCH, LP], f16, name="ref", tag="ref")
                    nc.sync.dma_start(rsb[:], zt[t])
                nbt = nbpool.tile([128, LP], f32, name="nbt", tag="nbt")
                nc.sync.dma_start(nbt[:], nb[t])

                for i in range(2):
                    if t == i:   # self image: skip
                        continue
                    for qb in range(NQB):
                        prev = None
                        for ci, (r0, w) in enumerate(CHUNKS):
                            pt = psum.tile([128, 512], f32, name="qr", tag="qr")
                            for k in range(KCH):
                                nc.tensor.matmul(
                                    pt[:, :w],
                                    lhsT=qsb[i][:, k, qb * 128:(qb + 1) * 128],
                                    rhs=rsb[:, k, r0:r0 + w],
                                    start=(k == 0),
                                    stop=(k == KCH - 1),
                                )
                            scr = scrpool.tile([128, 512], f32, name="scr", tag="scr")
                            nc.vector.tensor_tensor(
                                scr[:, :w], pt[:, :w], nbt[:, r0:r0 + w],
                                op=Alu.subtract)
                            cm = smpool.tile([128, 1], f32, name="cmin", tag="cmin")
                            nc.vector.tensor_reduce(
                                cm[:], scr[:, :w], axis=AxX, op=Alu.max)
                            if ci == 0:
                                prev = cm
                            elif ci < len(CHUNKS) - 1:
                                nx = smpool.tile([128, 1], f32, name="nx", tag="nx")
                                nc.vector.tensor_tensor(
                                    nx[:], prev[:], cm[:], op=Alu.max)
                                prev = nx
                            else:
                                nc.vector.tensor_tensor(
                                    msb[i][qb][:, t:t + 1], prev[:], cm[:],
                                    op=Alu.max)

            # tail: per (img, qblock) extract 4 smallest, sqrt(x+|q|^2), mean
            for i in range(2):
                for qb in range(NQB):
                    m = msb[i][qb]
                    dsum = None
                    for it in range(4):
                        rmin = smpool.tile([128, 1], f32, name="rmin", tag="rmin")
                        nc.vector.tensor_reduce(rmin[:], m[:], axis=AxX, op=Alu.max)
                        if it < 3:
                            mask = smpool.tile([128, N], f32, name="mask", tag="mask")
                            nc.vector.tensor_scalar(
                                out=mask[:], in0=m[:],
                                scalar1=rmin[:], scalar2=-BIG,
                                op0=Alu.is_equal, op1=Alu.mult,
                            )
                            nc.vector.tensor_tensor(m[:], m[:], mask[:], op=Alu.add)
                        d = smpool.tile([128, 1], f32, name="dist", tag="dist")
                        nc.scalar.activation(d[:], rmin[:], Sqrt,
                                             bias=q2sb[i][:, qb:qb + 1], scale=-2.0)
                        if dsum is None:
                            dsum = d
                        else:
                            s = smpool.tile([128, 1], f32, name="dsum", tag="dsum")
                            nc.vector.tensor_add(s[:], dsum[:], d[:])
                            dsum = s
                    nc.vector.tensor_scalar_mul(
                        scoresb[i][:, qb:qb + 1], dsum[:], 0.25)

            for i in range(2):
                nc.sync.dma_start(out[i], scoresb[i][:])
    nc.compile()
    return nc


def _build2():
    """Phase 2: exact rescue. 64 candidate patches (4 per image, chosen by
    phase-1 scores) as M=64 stationary; each core computes the per-ref-image
    min over ITS OWN 2 images' refs, with the cross term at ~fp32 precision
    via a 3-term fp16 split (qh*rh + ql*rh + qh*rl) accumulated in PSUM."""
    import concourse.bacc as bacc
    import concourse.tile as tile
    from concourse import mybir

    f16 = mybir.dt.float16
    f32 = mybir.dt.float32
    Alu = mybir.AluOpType
    AxX = mybir.AxisListType.X
    NT = 24   # 3 terms x 8 k-chunks

    nc = bacc.Bacc("TRN2", target_bir_lowering=False, debug=False)
    qc = nc.dram_tensor("qc", [128, NT, 64], f16, kind="ExternalInput").ap()
    rh = nc.dram_tensor("rh", [2, 128, KCH, LP], f16, kind="ExternalInput").ap()
    rl = nc.dram_tensor("rl", [2, 128, KCH, LP], f16, kind="ExternalInput").ap()
    nb2 = nc.dram_tensor("nb2", [2, 128, LP], f32, kind="ExternalInput").ap()
    out = nc.dram_tensor("m2", [2, 64], f32, kind="ExternalOutput").ap()

    with tile.TileContext(nc) as tc:
        with (
            tc.tile_pool(name="p2", bufs=1) as p2,
            tc.tile_pool(name="ref2", bufs=2) as ref2,
            tc.tile_pool(name="sm2", bufs=8) as sm2,
            tc.tile_pool(name="scr2", bufs=4) as scr2,
            tc.tile_pool(name="ps2", bufs=6, space="PSUM") as ps2,
        ):
            qcs = p2.tile([128, NT, 64], f16, name="qcs")
            nc.sync.dma_start(qcs[:], qc[:])
            for pos in range(2):
                rhs_t = ref2.tile([128, KCH, LP], f16, name="rh_t", tag="rh_t")
                nc.sync.dma_start(rhs_t[:], rh[pos])
                rls_t = ref2.tile([128, KCH, LP], f16, name="rl_t", tag="rl_t")
                nc.sync.dma_start(rls_t[:], rl[pos])
                nbt = ref2.tile([128, LP], f32, name="nb_t", tag="nb_t")
                nc.sync.dma_start(nbt[:], nb2[pos])

                prev = None
                for ci, (r0, w) in enumerate(CHUNKS):
                    pt = ps2.tile([64, 512], f32, name="qr2", tag="qr2")
                    for t in range(NT):
                        src = rhs_t if t < 16 else rls_t
                        k = t % KCH
                        nc.tensor.matmul(
                            pt[:, :w],
                            lhsT=qcs[:, t, :],
                            rhs=src[:, k, r0:r0 + w],
                            start=(t == 0),
                            stop=(t == NT - 1),
                        )
                    scr = scr2.tile([64, 512], f32, name="scr_2", tag="scr_2")
                    nc.vector.tensor_tensor(
                        scr[:, :w], pt[:, :w], nbt[:64, r0:r0 + w], op=Alu.add)
                    cm = sm2.tile([64, 1], f32, name="cm2", tag="cm2")
                    nc.vector.tensor_reduce(cm[:], scr[:, :w], axis=AxX, op=Alu.min)
                    if prev is None:
                        prev = cm
                    else:
                        nx = sm2.tile([64, 1], f32, name="nx2", tag="nx2")
                        nc.vector.tensor_tensor(nx[:], prev[:], cm[:], op=Alu.min)
                        prev = nx
                nc.sync.dma_start(out[pos], prev[:])
    nc.compile()
    return nc


def _host_prep(Z):
    Zp = np.full((N, LP, C), PAD_VAL, dtype=np.float16)
    Zp[:, :L, :] = Z.astype(np.float16)
    # [j, p, k, r] = Zp[j, r, 128k+p]
    zt_all = np.ascontiguousarray(Zp.reshape(N, LP, KCH, 128).transpose(0, 3, 2, 1))
    # fp16 residual of the padded refs (pads are exact in fp16 -> residual 0)
    Zp32 = np.zeros((N, LP, C), dtype=np.float32)
    Zp32[:, :L, :] = Z
    Zp32[:, L:, :] = np.float32(PAD_VAL)
    Zlo = (Zp32 - Zp.astype(np.float32)).astype(np.float16)
    zl_all = np.ascontiguousarray(Zlo.reshape(N, LP, KCH, 128).transpose(0, 3, 2, 1))
    nr = (Z.astype(np.float64) ** 2).sum(-1)
    nrp = np.full((N, LP), PAD_NORM)
    nrp[:, :L] = nr
    nrp = nrp.astype(np.float32)
    return zt_all, zl_all, nrp


def _run_with_retry(nc, in_maps, trace, attempts=2):
    """One retry absorbs transient device-state failures (e.g. a poisoned
    exec unit left over from an unrelated crashed run)."""
    import time
    import concourse.bass_utils as bass_utils

    for a in range(attempts):
        try:
            return bass_utils.run_bass_kernel_spmd(
                nc, in_maps, core_ids=list(range(NCORES)), trace=trace)
        except Exception:
            if a == attempts - 1:
                raise
            time.sleep(5)


def kernel(Z, cls_tokens):
    Z = np.asarray(Z)
    cls_tokens = np.asarray(cls_tokens)

    if "nc" not in _CACHE:
        _CACHE["nc"] = _build()
    nc = _CACHE["nc"]

    zt_all, zl_all, nrp = _host_prep(Z)

    in_maps = []
    for c in range(NCORES):
        order = [(2 * c + t) % N for t in range(N)]
        zt_core = np.ascontiguousarray(zt_all[order])
        nb_core = np.ascontiguousarray(
            np.broadcast_to(0.5 * nrp[order][:, None, :], (N, 128, LP)).astype(np.float32))
        q2_core = np.ascontiguousarray(
            nrp[2 * c:2 * c + 2].reshape(2, NQB, 128).transpose(0, 2, 1))
        in_maps.append({"zt": zt_core, "nb": nb_core, "q2": q2_core})

    trace = bool(int(os.environ.get("KERNEL_TRACE", "0")))
    res = _run_with_retry(nc, in_maps, trace)
    _CACHE["last_results"] = res

    patch_scores = np.zeros((N, L), dtype=np.float64)
    for c in range(NCORES):
        sc = res.results[c]["scores"]          # [2, 128, NQB]
        flat = sc.transpose(0, 2, 1).reshape(2, LP)   # [2, qb*128+p]
        patch_scores[2 * c:2 * c + 2] = flat[:, :L]

    img = patch_scores.max(-1)

    if bool(int(os.environ.get("KERNEL_RESCUE", "1"))):
        img = _rescue(Z, patch_scores, zt_all, zl_all, nrp, trace)

    return _host_tail(img, cls_tokens)


def _rescue(Z, patch_scores, zt_all, zl_all, nrp, trace):
    """Phase 2: recompute the top-4 candidate patches per image at ~fp32
    precision on-device (sharded over ref images) and return exact image
    scores."""
    import concourse.bass_utils as bass_utils

    if "nc2" not in _CACHE:
        _CACHE["nc2"] = _build2()
    nc2 = _CACHE["nc2"]

    NT, P = 24, 4
    cand = np.argsort(-patch_scores, axis=-1)[:, :P]     # [16, 4]
    qidx = cand.reshape(-1)                              # m = img*4 + rank
    qimg = np.repeat(np.arange(N), P)
    qf32 = Z[qimg, qidx].astype(np.float32)              # [64, 1024]
    qs = -2.0 * qf32
    qh = qs.astype(np.float16)
    ql = (qs - qh.astype(np.float32)).astype(np.float16)
    # qc[p, t, m]: t 0-7 -> qh chunk t; 8-15 -> ql; 16-23 -> qh
    qc = np.zeros((128, NT, 64), dtype=np.float16)
    qh_t = qh.reshape(64, KCH, 128).transpose(2, 1, 0)   # [128, 8, 64]
    ql_t = ql.reshape(64, KCH, 128).transpose(2, 1, 0)
    qc[:, 0:8] = qh_t
    qc[:, 8:16] = ql_t
    qc[:, 16:24] = qh_t

    in_maps2 = []
    for c in range(NCORES):
        sel = [2 * c, 2 * c + 1]
        in_maps2.append({
            "qc": qc,
            "rh": zt_all[sel],
            "rl": zl_all[sel],
            "nb2": np.ascontiguousarray(
                np.broadcast_to(nrp[sel][:, None, :], (2, 128, LP))),
        })
    res2 = _run_with_retry(nc2, in_maps2, trace)
    _CACHE["last_results2"] = res2

    m2 = np.zeros((64, N))
    for c in range(NCORES):
        m2[:, 2 * c] = res2.results[c]["m2"][0]
        m2[:, 2 * c + 1] = res2.results[c]["m2"][1]

    q2c = (qf32.astype(np.float64) ** 2).sum(-1)
    d2 = np.maximum(m2 + q2c[:, None], 1e-12)
    d = np.sqrt(d2)
    d[np.arange(64), qimg] = np.inf
    cscore = np.sort(d, axis=-1)[:, :4].mean(-1)         # [64]
    return cscore.reshape(N, P).max(-1)


def _host_tail(img, cls_tokens):
    # ---- tiny tail on host (float64) ----
    s = (img - img.min()) / (img.max() - img.min())
    W = cls_tokens.astype(np.float64) @ cls_tokens.astype(np.float64).T
    outs = []
    for k in (1, 2, 3):
        thr = np.sort(W, axis=-1)[:, N - k][:, None]
        Wm = np.where(W >= thr, W, 0.0)
        P = Wm / Wm.sum(-1, keepdims=True)
        outs.append(P @ s)
    return np.stack(outs, -1).mean(-1).astype(np.float32)



# revision 2
# speedup vs baseline: 1.1041x; 1.1041x over previous
"""MuSc (Mutual Scoring) Trainium2 kernel — v2 (symmetric + fp8 DoubleRow).

Problem: nn_BatchMuSc — Z:[16,1369,1024] patch features, cls_tokens:[16,1024].
MSM: for each image i, per-patch score = mean of the 4 smallest per-image
min-distances (excluding self). Then image scores -> min-max norm -> MMO over
cls-token similarity.

v2 strategy (8 NeuronCores):
  - SYMMETRY: d(q,r) is symmetric, so each unordered image pair {a,b} is
    computed ONCE as a [1408q x 1408r] block and reduced along BOTH axes:
    free-axis min -> a's patches vs b; partition-axis min -> b's patches vs a.
    This halves the matmul work vs the data-parallel baseline.
    120 pairs = 15 rounds x 8 cores (round-robin 1-factorization of K16);
    each core gets one pair per round -> perfectly balanced SPMD.
  - fp8 e4m3 inputs with DoubleRow matmuls (2 k-subtiles per MM) ~2x the
    fp16 MM rate. Ranking noise is absorbed by a widened exact rescue
    (empirically the true-best patch stays within the top-2 of the fp8
    ranking; we rescue the top-8 per image).
  - The ref-side norm rides INSIDE the fp8 stream: feature rows 1021-1023 are
    repurposed as base-{64,8,1} fp8 digit rows of -0.5|r|^2 on the ref (rhs)
    variant, with the matching constants {64,8,1} on the query (lhsT) variant.
    So PSUM = q.r(1021 feats) - 0.5|r|^2 from 4 pure-fp8 DR matmuls per chunk
    (no aug matmul, no dtype switches). The 3 dropped features and the <=0.125
    digit error add noise well under the fp8 quantization noise; the exact
    rescue absorbs both. Query norms are exact f32 via the ACT bias.
  - Per query block: ONE 3-bank PSUM tile [128,1536]; per chunk 4 DR matmuls
    into its bank-aligned slice; then:
      DVE  one tensor_reduce max over all 1408 refs -> free-side min.
      ACT  one Identity(+qn bias) copy psum -> s2 fp16 (partition staging).
      DVE  tensor_tensor max: acc = max(acc, s2) across query blocks (fp16 2x).
      Pool partition_all_reduce(max) once per pair -> partition-side min.
    Host applies the -2x and outer sqrt: min d^2 = -2*max(psum) (+|q|^2).
  - Phase 2 (exact rescue): top-8 patches per image (=128 candidates, one
    partition block) recomputed at ~fp32 precision (3-term fp16 split),
    sharded over ref images (2 per core). Host does the tiny tail in f64.
"""

import os
import numpy as np
import ml_dtypes

N = 16            # images
L = 1369          # patches per image
C = 1024          # feature dim
NCORES = 8
LP = 1408         # padded patches (11 * 128)
NQB = 11          # query blocks of 128
KCH = 8           # contraction chunks of 128
R = 15            # pair rounds
CHUNKS = [(0, 512), (512, 512), (1024, 352)]   # covers 1369 real + 7 pad cols
LE = 1376         # elementwise width (mult of 32 for DVE 2x packed mode)
PAD_VAL = 2.0     # pad-row feature value; pad distances are huge -> never win
BIG = 3.0e38
NT = 16           # rescue: 2 terms (qh,ql vs rh) x 8 k-chunks
NCAND = 8         # rescued candidates per image (128 total = 1 block)

_CACHE = {}


def _pair_schedule():
    """sched[r][c] = (a, b): round-robin 1-factorization of K16."""
    sched = []
    for r in range(R):
        pairs = [(15, r)]
        for k in range(1, 8):
            pairs.append(((r + k) % R, (r - k) % R))
        sched.append(pairs)
    return sched


def _build1(fp8=True, rounds=R):
    import concourse.bacc as bacc
    import concourse.tile as tile
    from concourse import mybir, bass_isa

    f16 = mybir.dt.float16
    f32 = mybir.dt.float32
    dt_z = mybir.dt.float8e4 if fp8 else f16
    Alu = mybir.AluOpType
    Copy = mybir.ActivationFunctionType.Copy
    DR = mybir.MatmulPerfMode.DoubleRow

    nc = bacc.Bacc("TRN2", target_bir_lowering=False, debug=False)

    zta = nc.dram_tensor("zta", [rounds, 128, KCH, LP], dt_z, kind="ExternalInput").ap()
    ztb = nc.dram_tensor("ztb", [rounds, 128, KCH, LP], dt_z, kind="ExternalInput").ap()
    qna = nc.dram_tensor("qna", [rounds, 128, NQB], f32, kind="ExternalInput").ap()
    outf = nc.dram_tensor("outf", [rounds, 128, NQB], f32, kind="ExternalOutput").ap()
    outp = nc.dram_tensor("outp", [rounds, 1, LE], f32, kind="ExternalOutput").ap()

    Identity = mybir.ActivationFunctionType.Identity

    with tile.TileContext(nc) as tc:
        with (
            tc.tile_pool(name="zpool", bufs=4) as zpool,
            tc.tile_pool(name="qnpool", bufs=2) as qnpool,
            tc.tile_pool(name="accpool", bufs=2) as accpool,
            tc.tile_pool(name="s2pool", bufs=2) as s2pool,
            tc.tile_pool(name="outfpool", bufs=2) as outfpool,
            tc.tile_pool(name="prpool", bufs=2) as prpool,
            tc.tile_pool(name="psum", bufs=2, space="PSUM") as psum,
        ):
            for r in range(rounds):
                za = zpool.tile([128, KCH, LP], dt_z, name="za", tag="za")
                nc.sync.dma_start(za[:], zta[r])
                zb = zpool.tile([128, KCH, LP], dt_z, name="zb", tag="zb")
                nc.sync.dma_start(zb[:], ztb[r])
                qn = qnpool.tile([128, NQB], f32, name="qn", tag="qn")
                nc.sync.dma_start(qn[:], qna[r])

                acc = accpool.tile([128, LE], f16, name="acc", tag="acc")
                outf_t = outfpool.tile([128, NQB], f32, name="outf_t", tag="outf_t")

                for qb in range(NQB):
                    ptb = psum.tile([128, 1536], f32, name="ptb", tag="ptb")
                    for ci, (c0, w) in enumerate(CHUNKS):
                        if fp8:
                            for kp in range(KCH // 2):
                                nc.tensor.matmul(
                                    ptb[:, c0:c0 + w],
                                    lhsT=za[:, 2 * kp:2 * kp + 2, qb * 128:(qb + 1) * 128],
                                    rhs=zb[:, 2 * kp:2 * kp + 2, c0:c0 + w],
                                    start=(kp == 0),
                                    stop=(kp == KCH // 2 - 1),
                                    perf_mode=DR,
                                )
                        else:
                            for k in range(KCH):
                                nc.tensor.matmul(
                                    ptb[:, c0:c0 + w],
                                    lhsT=za[:, k, qb * 128:(qb + 1) * 128],
                                    rhs=zb[:, k, c0:c0 + w],
                                    start=(k == 0),
                                    stop=(k == KCH - 1),
                                )
                    # free side: one max over all 1369 refs (psum = q.r-0.5|r|^2)
                    nc.vector.tensor_reduce(
                        outf_t[:, qb:qb + 1], ptb[:, :LE],
                        axis=mybir.AxisListType.X, op=Alu.max)
                    # partition side staging: s2 = psum + (-0.5|q|^2), fp16;
                    # qb 0 writes the accumulator directly
                    if qb == 0:
                        s2 = acc
                    else:
                        s2 = s2pool.tile([128, LE], f16, name="s2", tag="s2")
                    nc.scalar.activation(
                        s2[:, :LE], ptb[:, :LE], Identity,
                        bias=qn[:, qb:qb + 1], scale=1.0)
                    if qb > 0:
                        nc.vector.tensor_tensor(
                            acc[:, :LE], acc[:, :LE], s2[:, :LE], op=Alu.max)

                pr = prpool.tile([128, LE], f32, name="pr", tag="pr")
                nc.gpsimd.partition_all_reduce(
                    pr[:], acc[:, :LE], channels=128, reduce_op=bass_isa.ReduceOp.max)
                nc.sync.dma_start(outp[r], pr[0:1, :])
                nc.sync.dma_start(outf[r], outf_t[:])
    nc.compile()
    return nc


def _build2():
    """Exact rescue: 128 candidate patches (8/image) as stationary; each core
    computes max_r(q.r - 0.5|r|^2) over ITS 2 images' refs at high precision
    (2-term fp16 split: (qh+ql).rh = f32-exact q against fp16 refs)."""
    import concourse.bacc as bacc
    import concourse.tile as tile
    from concourse import mybir

    f16 = mybir.dt.float16
    f32 = mybir.dt.float32
    Alu = mybir.AluOpType

    nc = bacc.Bacc("TRN2", target_bir_lowering=False, debug=False)
    # qc t-slots 0..15: 2-term split; slot 16 rows 0-1: ones (aug lhsT)
    qc = nc.dram_tensor("qc", [128, NT + 1, 128], f16, kind="ExternalInput").ap()
    rh = nc.dram_tensor("rh", [2, 128, KCH, LP], f16, kind="ExternalInput").ap()
    augr = nc.dram_tensor("augr", [2, 2, LP], f16, kind="ExternalInput").ap()
    m2 = nc.dram_tensor("m2", [128, 2], f32, kind="ExternalOutput").ap()

    with tile.TileContext(nc) as tc:
        with (
            tc.tile_pool(name="qpool2", bufs=1) as qpool2,
            tc.tile_pool(name="ref2", bufs=2) as ref2,
            tc.tile_pool(name="aug2", bufs=2) as aug2,
            tc.tile_pool(name="out2", bufs=1) as out2,
            tc.tile_pool(name="ps2", bufs=2, space="PSUM") as ps2,
        ):
            qcs = qpool2.tile([128, NT + 1, 128], f16, name="qcs")
            nc.sync.dma_start(qcs[:], qc[:])
            m2t = out2.tile([128, 2], f32, name="m2t")
            for pos in range(2):
                # per-k DMA so matmuls start as soon as each chunk lands
                rkt = []
                for k in range(KCH):
                    t_ = ref2.tile([128, LP], f16, name=f"rk{k}", tag=f"rk{k}")
                    nc.sync.dma_start(t_[:], rh[pos, :, k, :])
                    rkt.append(t_)
                rnt = aug2.tile([2, LP], f16, name="rnt", tag="rnt")
                nc.sync.dma_start(rnt[:], augr[pos])

                ptb = ps2.tile([128, 1536], f32, name="ptb2", tag="ptb2")
                for k in range(KCH):
                    for term in range(2):
                        for ci, (c0, w) in enumerate(CHUNKS):
                            nc.tensor.matmul(
                                ptb[:, c0:c0 + w],
                                lhsT=qcs[:, term * KCH + k, :],
                                rhs=rkt[k][:, c0:c0 + w],
                                start=(k == 0 and term == 0),
                                stop=False,
                            )
                # aug: add -0.5|r|^2 (hi/lo rows x ones lhsT)
                for ci, (c0, w) in enumerate(CHUNKS):
                    nc.tensor.matmul(
                        ptb[:, c0:c0 + w],
                        lhsT=qcs[0:2, NT, :],
                        rhs=rnt[:, c0:c0 + w],
                        start=False, stop=True,
                    )
                nc.vector.tensor_reduce(
                    m2t[:, pos:pos + 1], ptb[:, :LE],
                    axis=mybir.AxisListType.X, op=Alu.max)
            nc.sync.dma_start(m2[:], m2t[:])
    nc.compile()
    return nc


DIGIT_SCALES = (64.0, 8.0, 1.0)


def _digit_rows(v):
    """Decompose v (~[-2100, -400]) into base-{64,8,1} rows, last row e4m3."""
    d1 = np.round(v / 64.0)
    r1 = v - 64.0 * d1
    d2 = np.round(r1 / 8.0)
    d3 = r1 - 8.0 * d2
    return d1, d2, d3


def _host_prep(Z, fp8=True):
    """Quantized transposed tiles (a/b variants) + exact norms + qn bias.

    Feature rows 1021-1023 (p=125..127 of k-chunk 7) are repurposed:
    a-variant (lhsT) holds the constants {64, 8, 1}; b-variant (rhs) holds
    the base-{64,8,1} digit rows of -0.5|r|^2, so the DR stream itself
    computes q.r(1021 feats) - 0.5|r|^2.
    """
    Zp = np.full((N, LP, C), PAD_VAL, dtype=np.float32)
    Zp[:, :L, :] = Z
    qdt = ml_dtypes.float8_e4m3 if fp8 else np.float16
    Zq = Zp.astype(qdt)
    # [img, p, k, r] = Zq[img, r, 128k+p]
    zt = np.ascontiguousarray(Zq.reshape(N, LP, KCH, 128).transpose(0, 3, 2, 1))
    nrm = (Zp.astype(np.float64) ** 2).sum(-1)          # [N, LP] exact full norm
    zta = zt.copy()
    for j, s in enumerate(DIGIT_SCALES):
        zta[:, 125 + j, 7, :] = qdt(s)
    ztb = zt
    d1, d2, d3 = _digit_rows(-0.5 * nrm)
    ztb[:, 125, 7, :] = d1.astype(qdt)
    ztb[:, 126, 7, :] = d2.astype(qdt)
    ztb[:, 127, 7, :] = d3.astype(qdt)
    qna = np.ascontiguousarray(
        (-0.5 * nrm).astype(np.float32).reshape(N, NQB, 128).transpose(0, 2, 1))
    return zta, ztb, nrm, qna


def _host_prep2(Z):
    """Rescue ref data: fp16 refs + hi/lo -0.5|r|^2 aug rows."""
    Zp = np.full((N, LP, C), PAD_VAL, dtype=np.float32)
    Zp[:, :L, :] = Z
    Zh = Zp.astype(np.float16)
    rh = np.ascontiguousarray(Zh.reshape(N, LP, KCH, 128).transpose(0, 3, 2, 1))
    nrm = (Zp.astype(np.float64) ** 2).sum(-1)
    hn = -0.5 * nrm
    hi = hn.astype(np.float16)
    lo = (hn - hi.astype(np.float64)).astype(np.float16)
    augr = np.stack([hi, lo], axis=1).astype(np.float16)   # [N, 2, LP]
    return rh, augr


def _run_with_retry(nc, in_maps, trace, attempts=3):
    import time
    import traceback
    import concourse.bass_utils as bass_utils

    import jax
    jax.devices()   # force PJRT backend init before the NTFF profile hook

    for a in range(attempts):
        try:
            return bass_utils.run_bass_kernel_spmd(
                nc, in_maps, core_ids=list(range(NCORES)), trace=trace)
        except Exception:
            traceback.print_exc()
            if a == attempts - 1:
                raise
            time.sleep(5)


def kernel(Z, cls_tokens):
    Z = np.asarray(Z, dtype=np.float32)
    cls_tokens = np.asarray(cls_tokens)
    fp8 = bool(int(os.environ.get("KERNEL_FP8", "1")))
    trace = bool(int(os.environ.get("KERNEL_TRACE", "0")))

    if "nc1" not in _CACHE:
        _CACHE["nc1"] = _build1(fp8=fp8)
    nc1 = _CACHE["nc1"]

    zta_all, ztb_all, nrm, qna = _host_prep(Z, fp8=fp8)
    sched = _pair_schedule()

    in_maps = []
    for c in range(NCORES):
        aa = [sched[r][c][0] for r in range(R)]
        bb = [sched[r][c][1] for r in range(R)]
        in_maps.append({
            "zta": np.ascontiguousarray(zta_all[aa]),
            "ztb": np.ascontiguousarray(ztb_all[bb]),
            "qna": np.ascontiguousarray(qna[aa]),
        })

    res = _run_with_retry(nc1, in_maps, trace)
    _CACHE["last_results"] = res

    # assemble per-patch min-d^2 matrix [img, patch, other-img]
    # free side: psum held q.r - 0.5|r|^2 -> min d^2 = |q|^2 - 2*max
    # partition side: s2 also had -0.5|q|^2 -> min d^2 = -2*max
    m2d = np.full((N, L, N), np.inf)
    for c in range(NCORES):
        outf = res.results[c]["outf"]          # [R, 128, NQB]
        outp = res.results[c]["outp"]          # [R, 1, LE]
        for r in range(R):
            a, b = sched[r][c]
            va = outf[r].transpose(1, 0).reshape(LP)[:L]   # q = qb*128+p
            m2d[a, :, b] = nrm[a][:L] - 2.0 * va.astype(np.float64)
            m2d[b, :, a] = -2.0 * outp[r, 0, :L].astype(np.float64)
    d = np.sqrt(np.maximum(m2d, 1e-12))
    for i in range(N):
        d[i, :, i] = np.inf
    pscore = np.partition(d, 3, axis=-1)[:, :, :4].mean(-1)   # [N, L]

    img = _rescue(Z, pscore, trace)
    return _host_tail(img, cls_tokens)


def _rescue(Z, pscore, trace):
    if "nc2" not in _CACHE:
        _CACHE["nc2"] = _build2()
    nc2 = _CACHE["nc2"]

    cand = np.argsort(-pscore, axis=-1)[:, :NCAND]       # [16, 8]
    qidx = cand.reshape(-1)
    qimg = np.repeat(np.arange(N), NCAND)
    qf = Z[qimg, qidx].astype(np.float32)                # [128, 1024]
    qh = qf.astype(np.float16)
    ql = (qf - qh.astype(np.float32)).astype(np.float16)
    qcm = np.zeros((128, NT + 1, 128), dtype=np.float16)
    qh_t = qh.reshape(128, KCH, 128).transpose(2, 1, 0)  # [p, k, cand]
    ql_t = ql.reshape(128, KCH, 128).transpose(2, 1, 0)
    qcm[:, 0:8] = qh_t
    qcm[:, 8:16] = ql_t
    qcm[0:2, NT] = 1.0                                   # aug ones rows

    rh, augr = _host_prep2(Z)
    in_maps = []
    for c in range(NCORES):
        sel = [2 * c, 2 * c + 1]
        in_maps.append({
            "qc": qcm,
            "rh": np.ascontiguousarray(rh[sel]),
            "augr": np.ascontiguousarray(augr[sel]),
        })
    res2 = _run_with_retry(nc2, in_maps, trace)
    _CACHE["last_results2"] = res2

    v = np.zeros((128, N))
    for c in range(NCORES):
        v[:, 2 * c:2 * c + 2] = res2.results[c]["m2"]
    q2c = (qf.astype(np.float64) ** 2).sum(-1)
    d2 = np.maximum(q2c[:, None] - 2.0 * v, 1e-12)
    dc = np.sqrt(d2)
    dc[np.arange(128), qimg] = np.inf
    cscore = np.sort(dc, axis=-1)[:, :4].mean(-1)
    return cscore.reshape(N, NCAND).max(-1)


def _host_tail(img, cls_tokens):
    s = (img - img.min()) / (img.max() - img.min())
    W = cls_tokens.astype(np.float64) @ cls_tokens.astype(np.float64).T
    outs = []
    for k in (1, 2, 3):
        thr = np.sort(W, axis=-1)[:, N - k][:, None]
        Wm = np.where(W >= thr, W, 0.0)
        P = Wm / Wm.sum(-1, keepdims=True)
        outs.append(P @ s)
    return np.stack(outs, -1).mean(-1).astype(np.float32)
